# revision 8
# baseline (speedup 1.0000x reference)
"""Trainium2 kernel for the t-product GNN layer (nn_ATGCO_16303695856134).

Math: out = (IFFT_t( FFT_t(adj) @bin FFT_t(x) ) real) @f weight
Factorization:
  - length-16 real FFT/IFFT folded into tiny 16x16 real matmuls on host
    (part of shard packing; <2% of FLOPs);
  - weight folded into the B-side spectrum on host: Bw_k = B_k @ weight;
  - device does the dominant compute: per frequency bin k,
      F_k^T = Bw_k^T @ A_k^T  (complex, via 2-4 real bf16 matmul accums),
    sharded one batch per NeuronCore (8 batches -> 8 cores, no collectives).
Device tensors (per core), comps grouped per-bin [R0 | R1 I1 | ... | R7 I7 | R8]
so each bin is ONE coalesced DMA:
  Ain  [16, 128, 4, 512] bf16 : A^T spectra; dims (comp, j%128, j-chunk, i)
  Bin  [16, 128, 4, 256] bf16 : Bw spectra;  dims (comp, j%128, j-chunk, o)
  Fout [16, 2, 128, 512] bf16 : F^T spectra; dims (comp, o-chunk, o%128, i)
"""

import sys

if "/opt/trn_rl_repo" not in sys.path:
    sys.path.insert(0, "/opt/trn_rl_repo")

import ml_dtypes
import numpy as np

import concourse.bass as bass
import concourse.mybir as mybir
import concourse.tile as tile
from concourse import bacc
from concourse.bass_utils import run_bass_kernel_spmd

T = 16
NB = 9          # rfft bins of a length-16 real signal
N = 512         # nodes
FIN = 256       # in features
FOUT = 256      # out features
NCORES = 8

# comp order: R0, R1, I1, R2, I2, ..., R7, I7, R8  (grouped per bin)
PERM = [0] + [v for k in range(1, 8) for v in (k, 9 + k - 1)] + [8]
IPERM = np.argsort(PERM)
BIN_C0 = {0: 0, 8: 15}
for _k in range(1, 8):
    BIN_C0[_k] = 2 * _k - 1

_BUILT = None


def _dft_mats():
    t = np.arange(T)
    ang = 2.0 * np.pi * np.outer(t, np.arange(NB)) / T
    Wf = np.concatenate([np.cos(ang), -np.sin(ang[:, 1:8])], axis=1).astype(
        np.float32
    )  # [16 t, 16 comps]: Re k=0..8, Im k=1..7 (fft e^{-i} convention)
    rows = [
        (1.0 if kk in (0, 8) else 2.0) * np.cos(2.0 * np.pi * t * kk / T) / T
        for kk in range(NB)
    ]
    rows += [-2.0 * np.sin(2.0 * np.pi * t * kk / T) / T for kk in range(1, 8)]
    IW = np.stack(rows).astype(np.float32)  # [16 comps, 16 t]
    return Wf, IW


def _build():
    global _BUILT
    if _BUILT is not None:
        return _BUILT

    nc = bacc.Bacc("TRN2", target_bir_lowering=False, debug=False,
                   num_devices=NCORES)
    bf16 = mybir.dt.bfloat16
    f32 = mybir.dt.float32

    a_dram = nc.dram_tensor("Ain", [16, 128, 4, N], bf16, kind="ExternalInput")
    b_dram = nc.dram_tensor("Bin", [16, 128, 4, FOUT], bf16, kind="ExternalInput")
    f_dram = nc.dram_tensor("Fout", [16, 2, 128, N], bf16, kind="ExternalOutput")

    with tile.TileContext(nc) as tc:
        with (
            tc.tile_pool(name="apool", bufs=3) as apool,
            tc.tile_pool(name="bpool", bufs=3) as bpool,
            tc.tile_pool(name="negpool", bufs=2) as negpool,
            tc.tile_pool(name="pspool", bufs=8, space="PSUM") as pspool,
            tc.tile_pool(name="fspool", bufs=3) as fspool,
        ):
            for kk in range(NB):
                c0 = BIN_C0[kk]
                ncmp = 2 if 1 <= kk <= 7 else 1
                at = apool.tile([128, ncmp, 4, N], bf16)
                nc.sync.dma_start(
                    out=at[:],
                    in_=a_dram[c0:c0 + ncmp].rearrange("c p a i -> p c a i"),
                )
                bt = bpool.tile([128, ncmp, 4, FOUT], bf16)
                nc.sync.dma_start(
                    out=bt[:],
                    in_=b_dram[c0:c0 + ncmp].rearrange("c p a f -> p c a f"),
                )
                if ncmp == 2:
                    bneg = negpool.tile([128, 4, FOUT], bf16)
                    nc.vector.tensor_scalar_mul(bneg[:], bt[:, 1], -1.0)
                    # (b-comp AP, a-comp idx) term lists: F_Re, F_Im
                    groups = [
                        (0, [(bt[:, 0], 0), (bneg[:], 1)]),
                        (1, [(bt[:, 0], 1), (bt[:, 1], 0)]),
                    ]
                else:
                    groups = [(0, [(bt[:, 0], 0)])]

                fs = fspool.tile([128, ncmp, 2, N], bf16)
                for gi, terms in groups:
                    for oc in range(2):
                        ps = pspool.tile([128, N], f32)
                        nmm = len(terms) * 4
                        mi = 0
                        for (bap, ac) in terms:
                            for jc in range(4):
                                nc.tensor.matmul(
                                    ps[:],
                                    bap[:, jc, oc * 128:(oc + 1) * 128],
                                    at[:, ac, jc, :],
                                    start=(mi == 0),
                                    stop=(mi == nmm - 1),
                                )
                                mi += 1
                        nc.vector.tensor_copy(fs[:, gi, oc, :], ps[:])
                nc.scalar.dma_start(
                    out=f_dram[c0:c0 + ncmp].rearrange("c oc p i -> p c oc i"),
                    in_=fs[:],
                )

    nc.compile()
    _BUILT = nc
    return nc


def kernel(x, adj, weight):
    x = np.asarray(x, dtype=np.float32)
    adj = np.asarray(adj, dtype=np.float32)
    weight = np.asarray(weight, dtype=np.float32)
    B = adj.shape[0]
    Wf, IW = _dft_mats()

    # A side: adj[b,i,j,t] --DFT--> [b,c,j,i]; comp-grouped, partition-major
    Ah = (adj.reshape(-1, T) @ Wf).reshape(B, N, N, 16).transpose(0, 3, 2, 1)
    Ah = Ah[:, PERM].reshape(B, 16, 4, 128, N).transpose(0, 1, 3, 2, 4)
    Ah = np.ascontiguousarray(Ah).astype(ml_dtypes.bfloat16)

    # B side: x[b,j,f,t] --DFT--> [b,c,j,f] --@weight--> [b,c,j,o]
    Bh = (x.reshape(-1, T) @ Wf).reshape(B, N, FIN, 16).transpose(0, 3, 1, 2)
    Bw = (np.ascontiguousarray(Bh).reshape(-1, FIN) @ weight).reshape(
        B, 16, N, FOUT
    )
    Bw = Bw[:, PERM].reshape(B, 16, 4, 128, FOUT).transpose(0, 1, 3, 2, 4)
    Bpack = np.ascontiguousarray(Bw).astype(ml_dtypes.bfloat16)

    nc = _build()
    in_maps = [{"Ain": Ah[b], "Bin": Bpack[b]} for b in range(B)]
    res = run_bass_kernel_spmd(nc, in_maps, core_ids=list(range(NCORES))).results

    F = np.stack([r["Fout"] for r in res]).astype(np.float32)  # [b,16,2,128,N]
    F = F.reshape(B, 16, FOUT, N)[:, IPERM]                    # [b,c(R0..8,I1..7),o,i]
    out = (
        np.ascontiguousarray(F.transpose(0, 3, 2, 1)).reshape(-1, 16) @ IW
    ).reshape(B, N, FOUT, T)
    return out.astype(np.float32)
